# revision 3
# baseline (speedup 1.0000x reference)
"""Trainium2 Bass kernel for Bahdanau-style additive attention.

Reference computation (B=2, Q=512, K=1024, QD=KD=256, H=128):
    pq = queries @ Wq.T                              # (B,Q,H)
    pk = keys @ Wk.T                                 # (B,K,H)
    energy[b,q,k] = sum_h We[h] * tanh(pq[b,q,h] + pk[b,k,h])
    energy = where(mask==0, -1e9, energy)
    A = softmax(energy, axis=k)
    context = A @ keys                               # (B,Q,QD)
    returns (context, A)

Sharding: flatten (B,Q) -> 1024 query rows, 128 rows per core (8 cores).
Softmax along K is local per row -> no collectives. Each core receives its
query shard (transposed), its batch's keys (both layouts), its mask shard,
and the full (tiny) weights.

Per-core dataflow:
  - PE: pqT (h x q) and pkT (h x k) projections.
  - Main loop over q=0..127: ACT computes tanh(pkT + pq[:,q]) via the fused
    per-partition bias (h lives on partitions), output bf16; PE reduces over
    h with a stationary that has We placed in column q (sliding window of a
    (128 x 255) buffer), accumulating energy rows into PSUM at partition q.
  - Masked softmax on DVE (+one ACT exp), context via PE transposes of A and
    an accumulated matmul against the naturally-laid-out keys.
"""

import sys

sys.path.insert(0, "/opt/trn_rl_repo")

import numpy as np

import concourse.bass as bass
import concourse.tile as tile
from concourse import bacc, mybir
from concourse.bass_utils import run_bass_kernel_spmd
from concourse.masks import make_identity

F32 = mybir.dt.float32
BF16 = mybir.dt.bfloat16
I32 = mybir.dt.int32

B, Q, K = 2, 512, 1024
QD, KD, H = 256, 256, 128
P = 128  # queries per core == partitions == hidden
N_CORES = 8
CORES_PER_BATCH = Q // P  # 4


def build_graph():
    """Build the single-core Bass graph (SPMD across 8 cores)."""
    nc = bacc.Bacc("TRN2", debug=False, num_devices=N_CORES)

    qT_d = nc.dram_tensor("qT", [2, 128, P], F32, kind="ExternalInput")
    wqT_d = nc.dram_tensor("wqT", [2, 128, H], F32, kind="ExternalInput")
    wkT_d = nc.dram_tensor("wkT", [2, 128, H], F32, kind="ExternalInput")
    keysT_d = nc.dram_tensor("keysT", [2, 128, K], F32, kind="ExternalInput")
    keys_d = nc.dram_tensor("keys", [8, 128, KD], F32, kind="ExternalInput")
    we_d = nc.dram_tensor("we", [H, 1], F32, kind="ExternalInput")
    mask_d = nc.dram_tensor("mask", [P, K], I32, kind="ExternalInput")
    ctx_d = nc.dram_tensor("ctx", [P, QD], F32, kind="ExternalOutput")
    attn_d = nc.dram_tensor("attn", [P, K], F32, kind="ExternalOutput")

    Tanh = mybir.ActivationFunctionType.Tanh
    Exp = mybir.ActivationFunctionType.Exp

    with tile.TileContext(nc) as tc:
        with (
            tc.tile_pool(name="consts", bufs=1) as consts,
            tc.tile_pool(name="work", bufs=1) as work,
            tc.tile_pool(name="tq", bufs=4) as tqp,
            tc.tile_pool(name="psE", bufs=1, space=bass.MemorySpace.PSUM) as psE,
            tc.tile_pool(name="psm1", bufs=1, space=bass.MemorySpace.PSUM) as psm1,
            tc.tile_pool(name="psm2", bufs=2, space=bass.MemorySpace.PSUM) as psm2,
        ):
            # ---- loads -------------------------------------------------
            qT_s = consts.tile([128, 2, P], F32)
            wqT_s = consts.tile([128, 2, H], F32)
            wkT_s = consts.tile([128, 2, H], F32)
            for c in range(2):
                nc.sync.dma_start(out=qT_s[:, c, :], in_=qT_d[c])
                nc.sync.dma_start(out=wqT_s[:, c, :], in_=wqT_d[c])
                nc.sync.dma_start(out=wkT_s[:, c, :], in_=wkT_d[c])
            keysT_s = consts.tile([128, 2, K], F32)
            for c in range(2):
                nc.sync.dma_start(out=keysT_s[:, c, :], in_=keysT_d[c])
            keys_s = consts.tile([128, 8, KD], F32)
            for c in range(8):
                nc.sync.dma_start(out=keys_s[:, c, :], in_=keys_d[c])
            mask_s = consts.tile([P, K], I32)
            nc.sync.dma_start(out=mask_s[:], in_=mask_d[:])
            we_s = consts.tile([H, 1], F32)
            nc.sync.dma_start(out=we_s[:], in_=we_d[:])

            # We placed at column 127 of a (128 x 255) zero buffer; slicing
            # [127-q : 255-q] yields a (128 x 128) stationary with We in
            # column q and zeros elsewhere.
            webig = consts.tile([H, 2 * P - 1], BF16)
            nc.vector.memset(webig[:], 0.0)
            nc.vector.tensor_copy(out=webig[:, P - 1 : P], in_=we_s[:])

            ident = consts.tile([128, 128], F32)
            make_identity(nc, ident[:])

            # ---- projections ------------------------------------------
            pq_ps = psm1.tile([H, P], F32, tag="pq")
            for c in range(2):
                nc.tensor.matmul(
                    out=pq_ps[:],
                    lhsT=wqT_s[:, c, :],
                    rhs=qT_s[:, c, :],
                    start=(c == 0),
                    stop=(c == 1),
                )
            pqT_s = work.tile([H, P], F32)
            nc.vector.tensor_copy(out=pqT_s[:], in_=pq_ps[:])

            pkT_s = work.tile([H, K], F32)
            for kh in range(2):
                pk_ps = psm2.tile([H, 512], F32, tag="pk")
                for c in range(2):
                    nc.tensor.matmul(
                        out=pk_ps[:],
                        lhsT=wkT_s[:, c, :],
                        rhs=keysT_s[:, c, 512 * kh : 512 * (kh + 1)],
                        start=(c == 0),
                        stop=(c == 1),
                    )
                nc.vector.tensor_copy(
                    out=pkT_s[:, 512 * kh : 512 * (kh + 1)], in_=pk_ps[:]
                )

            # ---- main loop: energy rows -------------------------------
            E0 = psE.tile([P, 512], F32, tag="E0")
            E1 = psE.tile([P, 512], F32, tag="E1")
            for q in range(P):
                tq = tqp.tile([H, K], BF16, tag="tq")
                nc.scalar.activation(
                    out=tq[:],
                    in_=pkT_s[:],
                    func=Tanh,
                    bias=pqT_s[:, q : q + 1],
                    scale=1.0,
                )
                st = webig[:, P - 1 - q : 2 * P - 1 - q]
                nc.tensor.matmul(
                    out=E0[:], lhsT=st, rhs=tq[:, 0:512],
                    start=(q == 0), stop=(q == P - 1),
                )
                nc.tensor.matmul(
                    out=E1[:], lhsT=st, rhs=tq[:, 512:1024],
                    start=(q == 0), stop=(q == P - 1),
                )

            # ---- masked softmax ---------------------------------------
            Em = work.tile([P, K], F32)
            nc.vector.memset(Em[:], -1e9)
            nc.vector.copy_predicated(out=Em[:, 0:512], mask=mask_s[:, 0:512], data=E0[:])
            nc.vector.copy_predicated(out=Em[:, 512:1024], mask=mask_s[:, 512:1024], data=E1[:])

            rowmax = work.tile([P, 1], F32)
            nc.vector.tensor_reduce(
                out=rowmax[:], in_=Em[:], axis=mybir.AxisListType.X,
                op=mybir.AluOpType.max,
            )
            negmax = work.tile([P, 1], F32)
            nc.vector.tensor_scalar_mul(negmax[:], rowmax[:], -1.0)
            Pexp = work.tile([P, K], F32)
            nc.scalar.activation(
                out=Pexp[:], in_=Em[:], func=Exp, bias=negmax[:], scale=1.0
            )
            rowsum = work.tile([P, 1], F32)
            nc.vector.tensor_reduce(
                out=rowsum[:], in_=Pexp[:], axis=mybir.AxisListType.X,
                op=mybir.AluOpType.add,
            )
            rinv = work.tile([P, 1], F32)
            nc.vector.reciprocal(out=rinv[:], in_=rowsum[:])
            A_s = work.tile([P, K], F32)
            nc.vector.tensor_scalar_mul(A_s[:], Pexp[:], rinv[:])
            nc.sync.dma_start(out=attn_d[:], in_=A_s[:])

            # ---- context = A @ keys -----------------------------------
            ctx_ps = psm1.tile([P, KD], F32, tag="ctx")
            for c in range(8):
                at_ps = psm2.tile([128, 128], F32, tag="at")
                nc.tensor.transpose(
                    out=at_ps[:], in_=A_s[:, 128 * c : 128 * (c + 1)],
                    identity=ident[:],
                )
                at_sb = work.tile([128, 128], F32, tag="at_sb")
                nc.vector.tensor_copy(out=at_sb[:], in_=at_ps[:])
                nc.tensor.matmul(
                    out=ctx_ps[:], lhsT=at_sb[:], rhs=keys_s[:, c, :],
                    start=(c == 0), stop=(c == 7),
                )
            ctx_sb = work.tile([P, QD], F32)
            nc.vector.tensor_copy(out=ctx_sb[:], in_=ctx_ps[:])
            nc.sync.dma_start(out=ctx_d[:], in_=ctx_sb[:])

    nc.compile()
    return nc


_NC_CACHE = None


def _get_graph():
    global _NC_CACHE
    if _NC_CACHE is None:
        _NC_CACHE = build_graph()
    return _NC_CACHE


def make_in_maps(queries, keys, mask, Wq, Wk, We):
    """Host-side sharding: build the 8 per-core input dicts."""
    queries = np.asarray(queries, dtype=np.float32)
    keys = np.asarray(keys, dtype=np.float32)
    mask = np.asarray(mask, dtype=np.int32)
    Wq = np.asarray(Wq, dtype=np.float32)
    Wk = np.asarray(Wk, dtype=np.float32)
    We = np.asarray(We, dtype=np.float32)

    wqT = np.ascontiguousarray(Wq.T).reshape(2, 128, H)
    wkT = np.ascontiguousarray(Wk.T).reshape(2, 128, H)
    we2 = np.ascontiguousarray(We.reshape(H, 1))
    keysT_b = [
        np.ascontiguousarray(keys[b].T).reshape(2, 128, K) for b in range(B)
    ]
    keysN_b = [np.ascontiguousarray(keys[b].reshape(8, 128, KD)) for b in range(B)]

    in_maps = []
    for core in range(N_CORES):
        b, qi = divmod(core, CORES_PER_BATCH)
        q0 = qi * P
        qT = np.ascontiguousarray(queries[b, q0 : q0 + P].T).reshape(2, 128, P)
        in_maps.append(
            {
                "qT": qT,
                "wqT": wqT,
                "wkT": wkT,
                "keysT": keysT_b[b],
                "keys": keysN_b[b],
                "we": we2,
                "mask": np.ascontiguousarray(mask[b, q0 : q0 + P]),
            }
        )
    return in_maps


def unshard(results):
    ctx = np.empty((B, Q, QD), dtype=np.float32)
    attn = np.empty((B, Q, K), dtype=np.float32)
    for core in range(N_CORES):
        b, qi = divmod(core, CORES_PER_BATCH)
        q0 = qi * P
        ctx[b, q0 : q0 + P] = results[core]["ctx"]
        attn[b, q0 : q0 + P] = results[core]["attn"]
    return ctx, attn


def kernel(queries, keys, mask, Wq, Wk, We):
    nc = _get_graph()
    in_maps = make_in_maps(queries, keys, mask, Wq, Wk, We)
    res = run_bass_kernel_spmd(nc, in_maps, core_ids=list(range(N_CORES)))
    return unshard(res.results)


# revision 6
# speedup vs baseline: 557.5628x; 557.5628x over previous
"""Trainium2 Bass kernel for Bahdanau-style additive attention.

Reference computation (B=2, Q=512, K=1024, QD=KD=256, H=128):
    pq = queries @ Wq.T                              # (B,Q,H)
    pk = keys @ Wk.T                                 # (B,K,H)
    energy[b,q,k] = sum_h We[h] * tanh(pq[b,q,h] + pk[b,k,h])
    energy = where(mask==0, -1e9, energy)
    A = softmax(energy, axis=k)
    context = A @ keys                               # (B,Q,QD)
    returns (context, A)

Sharding: flatten (B,Q) -> 1024 query rows, 128 rows per core (8 cores).
Softmax along K is local per row -> no collectives. Each core receives its
query shard (transposed), its batch's keys (both layouts), its mask shard,
and the full (tiny) weights.

Per-core dataflow:
  - PE: pqT (h x q) and pkT (h x k) projections.
  - Main loop over q=0..127: ACT computes tanh(pkT + pq[:,q]) via the fused
    per-partition bias (h lives on partitions), output bf16; PE reduces over
    h with a stationary that has We placed in column q (sliding window of a
    (128 x 255) buffer), accumulating energy rows into PSUM at partition q.
  - Masked softmax on DVE (+one ACT exp), context via PE transposes of A and
    an accumulated matmul against the naturally-laid-out keys.
"""

import contextlib
import sys

sys.path.insert(0, "/opt/trn_rl_repo")

import numpy as np

import concourse.bass as bass
import concourse.tile as tile
from concourse import bacc, mybir
from concourse.bass_utils import run_bass_kernel_spmd
from concourse.masks import make_identity

F32 = mybir.dt.float32
BF16 = mybir.dt.bfloat16
I32 = mybir.dt.int32

B, Q, K = 2, 512, 1024
QD, KD, H = 256, 256, 128
P = 128  # queries per core == partitions == hidden
N_CORES = 8
CORES_PER_BATCH = Q // P  # 4


def build_graph(reps: int = 1):
    """Build the single-core Bass graph (SPMD across 8 cores).

    reps > 1 wraps the whole body in a dynamic loop that recomputes the
    identical result `reps` times — used only for steady-state HW timing
    (wall-clock delta between two rep counts divides out dispatch overhead).
    """
    nc = bacc.Bacc("TRN2", debug=False, num_devices=N_CORES)

    qT_d = nc.dram_tensor("qT", [2, 128, P], F32, kind="ExternalInput")
    wqT_d = nc.dram_tensor("wqT", [2, 128, H], F32, kind="ExternalInput")
    wkT_d = nc.dram_tensor("wkT", [2, 128, H], F32, kind="ExternalInput")
    keysT_d = nc.dram_tensor("keysT", [2, 128, K], F32, kind="ExternalInput")
    keys_d = nc.dram_tensor("keys", [8, 128, KD], F32, kind="ExternalInput")
    we_d = nc.dram_tensor("we", [H, 1], F32, kind="ExternalInput")
    mask_d = nc.dram_tensor("mask", [P, K], I32, kind="ExternalInput")
    ctx_d = nc.dram_tensor("ctx", [P, QD], F32, kind="ExternalOutput")
    attn_d = nc.dram_tensor("attn", [P, K], F32, kind="ExternalOutput")

    Tanh = mybir.ActivationFunctionType.Tanh
    Exp = mybir.ActivationFunctionType.Exp

    with tile.TileContext(nc) as tc:
        with (
            tc.tile_pool(name="consts", bufs=1) as consts,
            tc.tile_pool(name="work", bufs=1) as work,
            tc.tile_pool(name="tq", bufs=4) as tqp,
            tc.tile_pool(name="psE", bufs=1, space=bass.MemorySpace.PSUM) as psE,
            tc.tile_pool(name="psm1", bufs=1, space=bass.MemorySpace.PSUM) as psm1,
            tc.tile_pool(name="psm2", bufs=2, space=bass.MemorySpace.PSUM) as psm2,
        ):
            loop_cm = (
                tc.For_i(0, reps, 1, hint_engines=(mybir.EngineType.PE,))
                if reps > 1
                else contextlib.nullcontext()
            )
            with loop_cm:
                # ---- loads ---------------------------------------------
                qT_s = consts.tile([128, 2, P], F32)
                wqT_s = consts.tile([128, 2, H], F32)
                wkT_s = consts.tile([128, 2, H], F32)
                for c in range(2):
                    nc.sync.dma_start(out=qT_s[:, c, :], in_=qT_d[c])
                    nc.sync.dma_start(out=wqT_s[:, c, :], in_=wqT_d[c])
                    nc.sync.dma_start(out=wkT_s[:, c, :], in_=wkT_d[c])
                keysT_s = consts.tile([128, 2, K], F32)
                for c in range(2):
                    nc.sync.dma_start(out=keysT_s[:, c, :], in_=keysT_d[c])
                keys_s = consts.tile([128, 8, KD], F32)
                for c in range(8):
                    nc.sync.dma_start(out=keys_s[:, c, :], in_=keys_d[c])
                mask_s = consts.tile([P, K], I32)
                nc.sync.dma_start(out=mask_s[:], in_=mask_d[:])
                we_s = consts.tile([H, 1], F32)
                nc.sync.dma_start(out=we_s[:], in_=we_d[:])

                # We placed at column 127 of a (128 x 255) zero buffer;
                # slicing [127-q : 255-q] yields a (128 x 128) stationary
                # with We in column q and zeros elsewhere.
                webig = consts.tile([H, 2 * P - 1], BF16)
                nc.vector.memset(webig[:], 0.0)
                nc.vector.tensor_copy(out=webig[:, P - 1 : P], in_=we_s[:])

                ident = consts.tile([128, 128], F32)
                make_identity(nc, ident[:])

                # ---- projections ---------------------------------------
                pq_ps = psm1.tile([H, P], F32, tag="pq")
                for c in range(2):
                    nc.tensor.matmul(
                        out=pq_ps[:],
                        lhsT=wqT_s[:, c, :],
                        rhs=qT_s[:, c, :],
                        start=(c == 0),
                        stop=(c == 1),
                    )
                pqT_s = work.tile([H, P], F32)
                nc.vector.tensor_copy(out=pqT_s[:], in_=pq_ps[:])

                pkT_s = work.tile([H, K], F32)
                for kh in range(2):
                    pk_ps = psm2.tile([H, 512], F32, tag="pk")
                    for c in range(2):
                        nc.tensor.matmul(
                            out=pk_ps[:],
                            lhsT=wkT_s[:, c, :],
                            rhs=keysT_s[:, c, 512 * kh : 512 * (kh + 1)],
                            start=(c == 0),
                            stop=(c == 1),
                        )
                    nc.vector.tensor_copy(
                        out=pkT_s[:, 512 * kh : 512 * (kh + 1)], in_=pk_ps[:]
                    )

                # ---- main loop: energy rows ----------------------------
                E0 = psE.tile([P, 512], F32, tag="E0")
                E1 = psE.tile([P, 512], F32, tag="E1")
                for q in range(P):
                    tq = tqp.tile([H, K], BF16, tag="tq")
                    nc.scalar.activation(
                        out=tq[:],
                        in_=pkT_s[:],
                        func=Tanh,
                        bias=pqT_s[:, q : q + 1],
                        scale=1.0,
                    )
                    st = webig[:, P - 1 - q : 2 * P - 1 - q]
                    nc.tensor.matmul(
                        out=E0[:], lhsT=st, rhs=tq[:, 0:512],
                        start=(q == 0), stop=(q == P - 1),
                    )
                    nc.tensor.matmul(
                        out=E1[:], lhsT=st, rhs=tq[:, 512:1024],
                        start=(q == 0), stop=(q == P - 1),
                    )

                # ---- masked softmax ------------------------------------
                Em = work.tile([P, K], F32)
                nc.vector.memset(Em[:], -1e9)
                nc.vector.copy_predicated(
                    out=Em[:, 0:512], mask=mask_s[:, 0:512], data=E0[:]
                )
                nc.vector.copy_predicated(
                    out=Em[:, 512:1024], mask=mask_s[:, 512:1024], data=E1[:]
                )

                rowmax = work.tile([P, 1], F32)
                nc.vector.tensor_reduce(
                    out=rowmax[:], in_=Em[:], axis=mybir.AxisListType.X,
                    op=mybir.AluOpType.max,
                )
                negmax = work.tile([P, 1], F32)
                nc.vector.tensor_scalar_mul(negmax[:], rowmax[:], -1.0)
                Pexp = work.tile([P, K], F32)
                nc.scalar.activation(
                    out=Pexp[:], in_=Em[:], func=Exp, bias=negmax[:], scale=1.0
                )
                rowsum = work.tile([P, 1], F32)
                nc.vector.tensor_reduce(
                    out=rowsum[:], in_=Pexp[:], axis=mybir.AxisListType.X,
                    op=mybir.AluOpType.add,
                )
                rinv = work.tile([P, 1], F32)
                nc.vector.reciprocal(out=rinv[:], in_=rowsum[:])
                A_s = work.tile([P, K], F32)
                nc.vector.tensor_scalar_mul(A_s[:], Pexp[:], rinv[:])
                nc.sync.dma_start(out=attn_d[:], in_=A_s[:])

                # ---- context = A @ keys --------------------------------
                ctx_ps = psm1.tile([P, KD], F32, tag="ctx")
                for c in range(8):
                    at_ps = psm2.tile([128, 128], F32, tag="at")
                    nc.tensor.transpose(
                        out=at_ps[:], in_=A_s[:, 128 * c : 128 * (c + 1)],
                        identity=ident[:],
                    )
                    at_sb = work.tile([128, 128], F32, tag="at_sb")
                    nc.vector.tensor_copy(out=at_sb[:], in_=at_ps[:])
                    nc.tensor.matmul(
                        out=ctx_ps[:], lhsT=at_sb[:], rhs=keys_s[:, c, :],
                        start=(c == 0), stop=(c == 7),
                    )
                ctx_sb = work.tile([P, QD], F32)
                nc.vector.tensor_copy(out=ctx_sb[:], in_=ctx_ps[:])
                nc.sync.dma_start(out=ctx_d[:], in_=ctx_sb[:])

    nc.compile()
    return nc


_NC_CACHE = {}


def _get_graph(reps: int = 1):
    if reps not in _NC_CACHE:
        _NC_CACHE[reps] = build_graph(reps)
    return _NC_CACHE[reps]


def make_in_maps(queries, keys, mask, Wq, Wk, We):
    """Host-side sharding: build the 8 per-core input dicts."""
    queries = np.asarray(queries, dtype=np.float32)
    keys = np.asarray(keys, dtype=np.float32)
    mask = np.asarray(mask, dtype=np.int32)
    Wq = np.asarray(Wq, dtype=np.float32)
    Wk = np.asarray(Wk, dtype=np.float32)
    We = np.asarray(We, dtype=np.float32)

    wqT = np.ascontiguousarray(Wq.T).reshape(2, 128, H)
    wkT = np.ascontiguousarray(Wk.T).reshape(2, 128, H)
    we2 = np.ascontiguousarray(We.reshape(H, 1))
    keysT_b = [
        np.ascontiguousarray(keys[b].T).reshape(2, 128, K) for b in range(B)
    ]
    keysN_b = [np.ascontiguousarray(keys[b].reshape(8, 128, KD)) for b in range(B)]

    in_maps = []
    for core in range(N_CORES):
        b, qi = divmod(core, CORES_PER_BATCH)
        q0 = qi * P
        qT = np.ascontiguousarray(queries[b, q0 : q0 + P].T).reshape(2, 128, P)
        in_maps.append(
            {
                "qT": qT,
                "wqT": wqT,
                "wkT": wkT,
                "keysT": keysT_b[b],
                "keys": keysN_b[b],
                "we": we2,
                "mask": np.ascontiguousarray(mask[b, q0 : q0 + P]),
            }
        )
    return in_maps


def unshard(results):
    ctx = np.empty((B, Q, QD), dtype=np.float32)
    attn = np.empty((B, Q, K), dtype=np.float32)
    for core in range(N_CORES):
        b, qi = divmod(core, CORES_PER_BATCH)
        q0 = qi * P
        ctx[b, q0 : q0 + P] = results[core]["ctx"]
        attn[b, q0 : q0 + P] = results[core]["attn"]
    return ctx, attn


def kernel(queries, keys, mask, Wq, Wk, We):
    nc = _get_graph()
    in_maps = make_in_maps(queries, keys, mask, Wq, Wk, We)
    res = run_bass_kernel_spmd(nc, in_maps, core_ids=list(range(N_CORES)))
    return unshard(res.results)


# revision 20
# speedup vs baseline: 569.4518x; 1.0213x over previous
"""Trainium2 Bass kernel for Bahdanau-style additive attention.

Reference computation (B=2, Q=512, K=1024, QD=KD=256, H=128):
    pq = queries @ Wq.T                              # (B,Q,H)
    pk = keys @ Wk.T                                 # (B,K,H)
    energy[b,q,k] = sum_h We[h] * tanh(pq[b,q,h] + pk[b,k,h])
    energy = where(mask==0, -1e9, energy)
    A = softmax(energy, axis=k)
    context = A @ keys                               # (B,Q,QD)
    returns (context, A)

Sharding: flatten (B,Q) -> 1024 query rows, 128 rows per core (8 cores).
Softmax along K is local per row -> no collectives. Each core receives its
query shard (transposed), its batch's keys (both layouts), its mask shard,
and the full (tiny) weights.

Per-core dataflow (ACT tanh is the roofline: 16.8M evals / 128 lanes /
1.2 GHz ~ 109 us):
  - PE projects pqT (h x q) and pkT (h x k).
  - Main loop, 16 blocks of G=8 queries: DVE broadcast-adds
    S[:,j,:] = pkT + pq[:,q] (tensor_scalar, 2x mode), one big ACT tanh per
    block (bf16 out, amortizes the per-instruction overhead), PE reduces
    each query over h with a stationary that has We placed in column q
    (sliding window of a (128 x 255) zero buffer), accumulating energy rows
    into PSUM at partition q.
  - Queries are grouped into 4 accumulation groups of 32 rows (double-
    buffered 2-bank PSUM tiles); each group's softmax+context interleaves
    with the next group's tanh blocks, so only the last group's epilogue
    sits on the tail.
  - Softmax without max-subtraction (|energy| <= sum|We| ~ 9, exp is safe
    in f32; reference softmax is shift-invariant): ACT exp straight from
    PSUM, DVE tensor_tensor_reduce applies the mask and row-sums in one op,
    reciprocal + scale gives A. Context: PE transposes of A slabs feed an
    accumulated matmul against the naturally-laid-out keys.
"""

import contextlib
import sys

sys.path.insert(0, "/opt/trn_rl_repo")

import numpy as np

import concourse.bass as bass
import concourse.tile as tile
from concourse import bacc, mybir
from concourse.bass_utils import run_bass_kernel_spmd
from concourse.masks import make_identity

F32 = mybir.dt.float32
BF16 = mybir.dt.bfloat16
I32 = mybir.dt.int32

B, Q, K = 2, 512, 1024
QD, KD, H = 256, 256, 128
P = 128  # queries per core == partitions == hidden
N_CORES = 8
CORES_PER_BATCH = Q // P  # 4
G = 8  # queries per tanh block
QG = 32  # queries per PSUM accumulation group


def build_graph(reps: int = 1):
    """Build the single-core Bass graph (SPMD across 8 cores).

    reps > 1 wraps the whole body in a dynamic loop that recomputes the
    identical result `reps` times — used only for steady-state HW timing
    (wall-clock delta between two rep counts divides out dispatch overhead).
    """
    nc = bacc.Bacc("TRN2", debug=False, num_devices=N_CORES, num_swdge_queues=4)

    qT_d = nc.dram_tensor("qT", [2, 128, P], F32, kind="ExternalInput")
    wqT_d = nc.dram_tensor("wqT", [2, 128, H], F32, kind="ExternalInput")
    wkT_d = nc.dram_tensor("wkT", [2, 128, H], F32, kind="ExternalInput")
    keysT_d = nc.dram_tensor("keysT", [2, 128, K], F32, kind="ExternalInput")
    keys_d = nc.dram_tensor("keys", [8, 128, KD], F32, kind="ExternalInput")
    we_d = nc.dram_tensor("we", [H, 1], F32, kind="ExternalInput")
    mask_d = nc.dram_tensor("mask", [P, K], I32, kind="ExternalInput")
    ctx_d = nc.dram_tensor("ctx", [P, QD], F32, kind="ExternalOutput")
    attn_d = nc.dram_tensor("attn", [P, K], F32, kind="ExternalOutput")

    Tanh = mybir.ActivationFunctionType.Tanh
    Exp = mybir.ActivationFunctionType.Exp

    with tile.TileContext(nc) as tc:
        with (
            tc.tile_pool(name="consts", bufs=1) as consts,
            tc.tile_pool(name="work", bufs=1) as work,
            tc.tile_pool(name="sp", bufs=2) as sp,
            tc.tile_pool(name="tp", bufs=3) as tp,
            tc.tile_pool(name="psE", bufs=2, space=bass.MemorySpace.PSUM) as psE,
            tc.tile_pool(name="psm1", bufs=2, space=bass.MemorySpace.PSUM) as psm1,
            tc.tile_pool(name="psm2", bufs=2, space=bass.MemorySpace.PSUM) as psm2,
        ):
            loop_cm = (
                tc.For_i(0, reps, 1, hint_engines=(mybir.EngineType.PE,))
                if reps > 1
                else contextlib.nullcontext()
            )
            with loop_cm:
                # ---- loads (keysT first: it gates pk -> adds -> tanh) --
                keysT_s = consts.tile([128, 2, K], F32)
                for c in range(2):
                    for kq in range(2):
                        nc.sync.dma_start(
                            out=keysT_s[:, c, 512 * kq : 512 * (kq + 1)],
                            in_=keysT_d[c, :, 512 * kq : 512 * (kq + 1)],
                        )
                qT_s = consts.tile([128, 2, P], F32)
                wqT_s = consts.tile([128, 2, H], F32)
                wkT_s = consts.tile([128, 2, H], F32)
                for c in range(2):
                    nc.sync.dma_start(out=wkT_s[:, c, :], in_=wkT_d[c])
                    nc.sync.dma_start(out=qT_s[:, c, :], in_=qT_d[c])
                    nc.sync.dma_start(out=wqT_s[:, c, :], in_=wqT_d[c])
                we_s = consts.tile([H, 1], F32)
                nc.sync.dma_start(out=we_s[:], in_=we_d[:])
                keys_s = consts.tile([128, 8, KD], F32)
                for c in range(8):
                    nc.sync.dma_start(out=keys_s[:, c, :], in_=keys_d[c])
                mask_s = consts.tile([P, K], I32)
                nc.sync.dma_start(out=mask_s[:], in_=mask_d[:])

                # ACT table warmup: pull the tanh/exp table load into the
                # DMA window instead of the first real tanh.
                dummy = work.tile([H, 1], F32)
                nc.scalar.activation(out=dummy[:], in_=we_s[:], func=Tanh)

                # We placed at column 127 of a (128 x 255) zero buffer;
                # slicing [127-q : 255-q] yields a (128 x 128) stationary
                # with We in column q and zeros elsewhere.
                webig = consts.tile([H, 2 * P - 1], BF16)
                nc.vector.memset(webig[:], 0.0)
                nc.vector.tensor_copy(out=webig[:, P - 1 : P], in_=we_s[:])

                ident = consts.tile([128, 128], F32)
                make_identity(nc, ident[:])

                maskf = work.tile([P, K], BF16)
                nc.vector.tensor_copy(out=maskf[:], in_=mask_s[:])

                keys_bf = consts.tile([128, 8, KD], BF16)
                for c in range(8):
                    nc.vector.tensor_copy(out=keys_bf[:, c, :], in_=keys_s[:, c, :])

                # ---- projections (PSUM->SBUF copies on idle ACT) -------
                pq_ps = psm1.tile([H, P], F32, tag="mm_misc")
                for c in range(2):
                    nc.tensor.matmul(
                        out=pq_ps[:],
                        lhsT=wqT_s[:, c, :],
                        rhs=qT_s[:, c, :],
                        start=(c == 0),
                        stop=(c == 1),
                    )
                pqT_s = work.tile([H, P], F32)
                nc.scalar.copy(out=pqT_s[:], in_=pq_ps[:])

                pkT_s = work.tile([H, K], F32)
                for kh in range(2):
                    pk_ps = psm2.tile([H, 512], F32, tag="pe_tmp")
                    for c in range(2):
                        nc.tensor.matmul(
                            out=pk_ps[:],
                            lhsT=wkT_s[:, c, :],
                            rhs=keysT_s[:, c, 512 * kh : 512 * (kh + 1)],
                            start=(c == 0),
                            stop=(c == 1),
                        )
                    nc.scalar.copy(
                        out=pkT_s[:, 512 * kh : 512 * (kh + 1)], in_=pk_ps[:]
                    )

                # ---- softmax/context buffers ---------------------------
                # The whole softmax chain runs full-width (128 partitions)
                # every group: several DVE ops are custom-ucode and reject
                # partition offsets (ISA start_addr_active_channels), and
                # all of them are free-dim-bound anyway so the extra rows
                # are free. Rows outside the current group compute harmless
                # values whose consumers never read them.
                Pexp = work.tile([P, K], BF16)
                nc.vector.memset(Pexp[:], 0.0)
                Pm = work.tile([P, K], BF16)
                A_s = work.tile([P, K], F32)
                nc.vector.memset(A_s[:], 0.0)
                rowsum = work.tile([P, 1], F32)
                rinv = work.tile([P, 1], F32)

                def epilogue_group(gr, E_grp):
                    """Masked softmax + context for query rows
                    [QG*gr, QG*(gr+1)), living in the same partitions."""
                    sl = slice(QG * gr, QG * (gr + 1))
                    nc.scalar.activation(out=Pexp[:], in_=E_grp[:], func=Exp)
                    # Pm = Pexp * maskf (mask values exact in bf16), then
                    # rowsum = sum_k Pm. (tensor_tensor_reduce would fuse
                    # these but its custom DVE ucode faults at runtime here.)
                    nc.vector.tensor_mul(Pm[:], Pexp[:], maskf[:])
                    nc.vector.tensor_reduce(
                        out=rowsum[:], in_=Pm[:], axis=mybir.AxisListType.X,
                        op=mybir.AluOpType.add,
                    )
                    nc.vector.reciprocal(out=rinv[:], in_=rowsum[:])
                    nc.vector.tensor_scalar_mul(A_s[:], Pm[:], rinv[:])
                    nc.sync.dma_start(out=attn_d[sl, :], in_=A_s[sl, :])

                    # context rows: transpose full-width A slabs at partition
                    # base 0 (stale rows' output columns are never read); the
                    # group's columns slice the bf16 stationary for an
                    # accumulated matmul against bf16 keys.
                    ctx_ps = psm1.tile([QG, KD], F32, tag="mm_misc", name=f"ctx{gr}")
                    for c in range(8):
                        at_ps = psm2.tile([128, 128], F32, tag="pe_tmp", name=f"at{gr}_{c}")
                        nc.tensor.transpose(
                            out=at_ps[:],
                            in_=A_s[:, 128 * c : 128 * (c + 1)],
                            identity=ident[:],
                        )
                        at_sb = work.tile([128, 128], BF16, tag="at_sb")
                        nc.vector.tensor_copy(out=at_sb[:], in_=at_ps[:])
                        nc.tensor.matmul(
                            out=ctx_ps[:], lhsT=at_sb[:, sl], rhs=keys_bf[:, c, :],
                            start=(c == 0), stop=(c == 7),
                        )
                    ctx_sb = work.tile([QG, QD], F32, tag="ctx_sb")
                    nc.vector.tensor_copy(out=ctx_sb[:], in_=ctx_ps[:])
                    nc.sync.dma_start(out=ctx_d[sl, :], in_=ctx_sb[:])

                # ---- main loop: energy rows ----------------------------
                E_grp = None
                for g in range(P // G):
                    s_blk = sp.tile([H, G, K], F32, tag="s_blk")
                    for j in range(G):
                        q = g * G + j
                        nc.vector.tensor_scalar_add(
                            s_blk[:, j, :], pkT_s[:], pqT_s[:, q : q + 1]
                        )
                    t_blk = tp.tile([H, G, K], BF16, tag="t_blk")
                    nc.scalar.activation(out=t_blk[:], in_=s_blk[:], func=Tanh)
                    for j in range(G):
                        q = g * G + j
                        if q % QG == 0:
                            gr = q // QG
                            E_grp = psE.tile(
                                [P, K], F32, tag="E", name=f"E{gr}"
                            )
                        st = webig[:, P - 1 - q : 2 * P - 1 - q]
                        nc.tensor.matmul(
                            out=E_grp[:, 0:512], lhsT=st, rhs=t_blk[:, j, 0:512],
                            start=(q % QG == 0), stop=(q % QG == QG - 1),
                        )
                        nc.tensor.matmul(
                            out=E_grp[:, 512:1024], lhsT=st,
                            rhs=t_blk[:, j, 512:1024],
                            start=(q % QG == 0), stop=(q % QG == QG - 1),
                        )
                        if q % QG == QG - 1:
                            epilogue_group(q // QG, E_grp)

    nc.compile()
    return nc


_NC_CACHE = {}


def _get_graph(reps: int = 1):
    if reps not in _NC_CACHE:
        _NC_CACHE[reps] = build_graph(reps)
    return _NC_CACHE[reps]


def make_in_maps(queries, keys, mask, Wq, Wk, We):
    """Host-side sharding: build the 8 per-core input dicts."""
    queries = np.asarray(queries, dtype=np.float32)
    keys = np.asarray(keys, dtype=np.float32)
    mask = np.asarray(mask, dtype=np.int32)
    Wq = np.asarray(Wq, dtype=np.float32)
    Wk = np.asarray(Wk, dtype=np.float32)
    We = np.asarray(We, dtype=np.float32)

    wqT = np.ascontiguousarray(Wq.T).reshape(2, 128, H)
    wkT = np.ascontiguousarray(Wk.T).reshape(2, 128, H)
    we2 = np.ascontiguousarray(We.reshape(H, 1))
    keysT_b = [
        np.ascontiguousarray(keys[b].T).reshape(2, 128, K) for b in range(B)
    ]
    keysN_b = [np.ascontiguousarray(keys[b].reshape(8, 128, KD)) for b in range(B)]

    in_maps = []
    for core in range(N_CORES):
        b, qi = divmod(core, CORES_PER_BATCH)
        q0 = qi * P
        qT = np.ascontiguousarray(queries[b, q0 : q0 + P].T).reshape(2, 128, P)
        in_maps.append(
            {
                "qT": qT,
                "wqT": wqT,
                "wkT": wkT,
                "keysT": keysT_b[b],
                "keys": keysN_b[b],
                "we": we2,
                "mask": np.ascontiguousarray(mask[b, q0 : q0 + P]),
            }
        )
    return in_maps


def unshard(results):
    ctx = np.empty((B, Q, QD), dtype=np.float32)
    attn = np.empty((B, Q, K), dtype=np.float32)
    for core in range(N_CORES):
        b, qi = divmod(core, CORES_PER_BATCH)
        q0 = qi * P
        ctx[b, q0 : q0 + P] = results[core]["ctx"]
        attn[b, q0 : q0 + P] = results[core]["attn"]
    return ctx, attn


def kernel(queries, keys, mask, Wq, Wk, We):
    nc = _get_graph()
    in_maps = make_in_maps(queries, keys, mask, Wq, Wk, We)
    res = run_bass_kernel_spmd(nc, in_maps, core_ids=list(range(N_CORES)))
    return unshard(res.results)
